# revision 23
# baseline (speedup 1.0000x reference)
"""Multi-head self-attention + LayerNorm, sharded over 8 TRN2 NeuronCores.

Problem: x[4, 2048, 1024], 16 heads x 64 dim, causal attention, output
projection, LayerNorm.  Sharding: core c handles batch c//2 and head-group
c%2 (8 heads).  All 8 cores run one SPMD program; the output projection
produces partial sums which are pair-wise AllReduced on device, then each
core applies the final LayerNorm.  Host gathers batch b from core 2*b.

Dtypes: projections / QK^T / output projection run in float32r (fp32
container, mantissa rounded to 11 explicit bits; full PE rate at N>=256).
The BIR verifier requires f32r matmul operands to be produced "rounded":
DRAM inputs are pre-rounded on the host and declared f32r; on-chip operands
are produced by ACT/DVE ops with f32r output dtype.  The AV matmul
(softmax weights x V) runs in bf16: P is in [0,1] and V noise averages out.

The kernel is one software pipeline over 512-row t-tiles: project QKV for
tile tt -> attention for q-tile tt (needs only K/V tiles <= tt, thanks to
causality) -> output projection -> pair-AllReduce chunk -> LayerNorm.
Later tiles' projections/attention overlap earlier tiles' collectives and
LN, and the PE always has dense work (keeps the HAM clock-gate at 2.4 GHz).
Diagonal attention blocks are column-trimmed to the causal region
(S / exp / mask / AV all skip the dead columns).
"""

import numpy as np

import concourse.bass as bass
import concourse.mybir as mybir
import concourse.tile as tile
from concourse import bacc, library_config
from concourse.bass_utils import run_bass_kernel_spmd

# Problem constants (hardcoded per harness contract)
B, T, C = 4, 2048, 1024
H, D = 16, 64
HG = 2                 # head groups (cores per batch)
HPG = H // HG          # heads per group = 8
CG = C // HG           # channels per group = 512
SCALE = D ** -0.5      # 0.125
LN_EPS = 1e-5

QT = 512               # q tile (moving free dim)
KT = 128               # k tile (PE contraction tile)
XH = 256               # x-stream half-tile width
NQT = T // QT          # 4
NKC = T // KT          # 16
NIC = C // 128         # 8 input-channel chunks
NDC = CG // 128        # 4 output d-chunks per group

F32 = mybir.dt.float32
F32R = mybir.dt.float32r
BF16 = mybir.dt.bfloat16

REPLICA_GROUPS = [[0, 1], [2, 3], [4, 5], [6, 7]]


def build_program():
    """Build + compile the single-core SPMD Bass program. Returns (nc, io)."""
    nc = bacc.Bacc(
        "TRN2",
        target_bir_lowering=False,
        debug=False,
        enable_asserts=False,
        num_devices=8,
    )

    # ---- DRAM I/O ----  (f32r inputs are pre-rounded fp32 on the host)
    xT = nc.dram_tensor("xT", [C, T], F32R, kind="ExternalInput")
    wqT = nc.dram_tensor("wqT", [C, CG], F32R, kind="ExternalInput")
    wkT = nc.dram_tensor("wkT", [C, CG], F32R, kind="ExternalInput")
    wvT = nc.dram_tensor("wvT", [C, CG], F32R, kind="ExternalInput")
    wpT = nc.dram_tensor("wpT", [CG, C], F32R, kind="ExternalInput")
    gamma = nc.dram_tensor("gamma", [C], F32, kind="ExternalInput")
    beta = nc.dram_tensor("beta", [C], F32, kind="ExternalInput")
    # 4 diagonal-block masks [128 k x 512 q]: 1.0 where 128*j + k_r <= q_r
    masks = nc.dram_tensor("masks", [4, KT, QT], BF16, kind="ExternalInput")
    y_out = nc.dram_tensor("y", [T, C], F32, kind="ExternalOutput")

    with tile.TileContext(nc) as tc:
        _body(tc, xT, wqT, wkT, wvT, wpT, gamma, beta, masks, y_out)

    nc.compile()
    io = dict(inputs=["xT", "wqT", "wkT", "wvT", "wpT", "gamma", "beta", "masks"],
              output="y")
    return nc, io


def _body(tc, xT, wqT, wkT, wvT, wpT, gamma, beta, masks, y_out):
    nc = tc.nc

    # ---------- persistent SBUF ----------
    persist = tc.alloc_tile_pool(name="persist", bufs=1)
    # K^T in [128 part, d-chunk, t] layout; head h lives at partition rows
    # 64*(h%2) .. +64 of chunk h//2.
    kT_sb = persist.tile([128, NDC, T], F32R)
    # V in [t(128-chunks) part, k-chunk, head, 65] layout; col 64 is the ones
    # column providing the softmax denominator in the AV matmul.
    v_sb = persist.tile([128, NKC, HPG, 65], BF16)
    mask_sb = persist.tile([128, 4, QT], BF16)
    eps_sb = persist.tile([128, 1], F32)
    wq_sb = persist.tile([128, NIC, CG], F32R)
    wk_sb = persist.tile([128, NIC, CG], F32R)
    wv_sb = persist.tile([128, NIC, CG], F32R)
    wp_sb = persist.tile([128, NDC, C], F32R)
    gamma_sb = persist.tile([128, C], F32)
    beta_sb = persist.tile([128, C], F32)

    nc.vector.memset(eps_sb, LN_EPS)
    # f32r/bf16 matmul operands cannot be memset directly; round via a copy
    ones_f = persist.tile([128, 128], F32)
    ones_sb = persist.tile([65, 64], F32R)
    nc.vector.memset(ones_f, 1.0)
    nc.scalar.copy(ones_sb, ones_f[0:65, 0:64])
    # ones columns of V
    nc.scalar.copy(
        v_sb[:, :, :, 64],
        ones_f[:, 0:NKC * HPG].rearrange("p (a b) -> p a b", a=NKC),
    )

    with (
        tc.tile_pool(name="qrot", bufs=2) as qpool,
        tc.tile_pool(name="xstream", bufs=2) as xpool,
        tc.tile_pool(name="psP", bufs=2, space="PSUM") as psP,
        tc.tile_pool(name="psS", bufs=2, space="PSUM") as psS,
        tc.tile_pool(name="psO", bufs=2, space="PSUM") as psO,
        tc.tile_pool(name="pT", bufs=3) as ppool,
        tc.tile_pool(name="recipp", bufs=2) as rpool,
        tc.tile_pool(name="bcastp", bufs=1) as bpool,
        tc.tile_pool(name="onormp", bufs=2) as opool,
        tc.tile_pool(name="ytile", bufs=4) as ypool,
        tc.tile_pool(name="lnst", bufs=3) as lnpool,
        tc.tile_pool(name="dram", bufs=1, space="DRAM") as dram,
    ):
        y_parts = [dram.tile([QT, C], F32, name=f"y_part{i}") for i in range(NQT)]
        y_reds = [dram.tile([QT // 2, C], F32, name=f"y_red{i}") for i in range(2 * NQT)]

        # weights on the SP queue; first x half-tiles in parallel on ACT queue.
        # Per-chunk DMAs let the first projection matmuls start as soon as
        # chunk 0 lands instead of waiting for the full tensors.
        wkv = wkT.ap().rearrange("(a p) o -> p a o", p=128)
        for ic in range(NIC):
            nc.sync.dma_start(out=wk_sb[:, ic, :], in_=wkv[:, ic, :])
        x_first = []
        for xh in range(2):
            x_h = xpool.tile([128, NIC, XH], F32R, name="x_h")
            xv = xT.ap()[:, xh * XH:(xh + 1) * XH].rearrange("(a p) t -> p a t", p=128)
            for ic in range(NIC):
                nc.scalar.dma_start(out=x_h[:, ic, :], in_=xv[:, ic, :])
            x_first.append(x_h)
        nc.sync.dma_start(out=wq_sb, in_=wqT.ap().rearrange("(a p) o -> p a o", p=128))
        nc.sync.dma_start(out=wv_sb, in_=wvT.ap().rearrange("(a p) o -> p a o", p=128))
        nc.sync.dma_start(out=mask_sb, in_=masks.ap().rearrange("j k q -> k j q"))
        nc.sync.dma_start(out=wp_sb, in_=wpT.ap().rearrange("(a p) o -> p a o", p=128))
        nc.gpsimd.dma_start(out=gamma_sb, in_=gamma.ap().unsqueeze(0).to_broadcast([128, C]))
        nc.gpsimd.dma_start(out=beta_sb, in_=beta.ap().unsqueeze(0).to_broadcast([128, C]))

        def project(tt, x_halves, qT_t):
            for xh, x_h in enumerate(x_halves):
                t0 = tt * QT + xh * XH
                # K^T and Q^T: out[d_chunk, t] = sum_i W[d, i] * xT[i, t]
                for w_sb, dst, dsl in (
                    (wk_sb, kT_sb, slice(t0, t0 + XH)),
                    (wq_sb, qT_t, slice(xh * XH, xh * XH + XH)),
                ):
                    for dc in range(NDC):
                        ps = psP.tile([128, XH], F32, tag="ps", name="ps_p")
                        for ic in range(NIC):
                            nc.tensor.matmul(
                                ps,
                                w_sb[:, ic, dc * 128:(dc + 1) * 128],
                                x_h[:, ic, :],
                                start=(ic == 0), stop=(ic == NIC - 1),
                            )
                        nc.vector.tensor_copy(dst[:, dc, dsl], ps)
                # V: out[t(128) part, d] = sum_i xT[i, t] * WvT[i, d]
                for j in range(XH // 128):
                    kc = tt * (QT // 128) + xh * (XH // 128) + j
                    ps = psP.tile([128, CG], F32, tag="ps", name="ps_v")
                    for ic in range(NIC):
                        nc.tensor.matmul(
                            ps,
                            x_h[:, ic, j * 128:(j + 1) * 128],
                            wv_sb[:, ic, :],
                            start=(ic == 0), stop=(ic == NIC - 1),
                        )
                    nc.vector.tensor_copy(
                        v_sb[:, kc, :, 0:64],
                        ps.rearrange("p (h d) -> p h d", h=HPG),
                    )

        def attention(hp, qt, qT_t):
            nkt = (qt + 1) * (QT // KT)  # causal k-extent in 128-tiles

            def trim(kt):
                # first valid q column of this k-tile within the q-tile
                diag = kt - (nkt - 4)
                return (0 if diag < 0 else diag * KT), diag

            ps_o = [psO.tile([65, QT], F32, tag="pso", name=f"ps_o{e}")
                    for e in range(2)]

            def s_pair(kt):
                # S^T[k, q] = K[k, :] . Q[q, :]; heads 2hp / 2hp+1 live on
                # partition rows 0-63 / 64-127 -> row-tiled, run concurrent
                off, _ = trim(kt)
                ps_s = psS.tile([128, 2, QT], F32, tag="pss", name="ps_s")
                for e in range(2):
                    r0, r1 = 64 * e, 64 * e + 64
                    nc.tensor.matmul(
                        ps_s[:, e, off:],
                        kT_sb[r0:r1, hp, kt * KT:(kt + 1) * KT],
                        qT_t[r0:r1, hp, off:],
                        start=True, stop=True,
                    )
                return ps_s

            # software-pipelined: S-pair for kt+1 issues before AV of kt
            ps_s_cur = s_pair(0)
            for kt in range(nkt):
                ps_s_next = s_pair(kt + 1) if kt + 1 < nkt else None
                off, diag = trim(kt)
                p_t = ppool.tile([128, 2, QT], BF16, tag="pt")
                # exp over both heads' tiles in one ACT call (trimmed)
                nc.scalar.activation(
                    p_t[:, :, off:], ps_s_cur[:, :, off:],
                    mybir.ActivationFunctionType.Exp, scale=SCALE,
                )
                if diag >= 0:
                    for e in range(2):
                        nc.vector.tensor_mul(
                            p_t[:, e, off:], p_t[:, e, off:],
                            mask_sb[:, diag, off:],
                        )
                for e in range(2):
                    h = 2 * hp + e
                    nc.tensor.matmul(
                        ps_o[e][:, off:],
                        v_sb[:, kt, h, :],
                        p_t[:, e, off:],
                        start=(kt == 0), stop=(kt == nkt - 1),
                    )
                ps_s_cur = ps_s_next

            # normalize: O^T[d, q] /= denom[q]; write into qT_t (=out^T).
            # denom row -> f32r, broadcast across 64 partitions via a ones
            # outer-product on the PE, approx-reciprocal, then one DVE
            # multiply per head.
            d_r = rpool.tile([65, 2, QT], F32R, tag="recip")
            rb = bpool.tile([64, 2, QT], F32, tag="rbcast")
            for e in range(2):
                nc.vector.tensor_copy(d_r[64:65, e, :], ps_o[e][64:65, :])
                db_ps = psP.tile([64, QT], F32, tag="ps", name="db_ps")
                nc.tensor.matmul(
                    db_ps, ones_sb[64:65, :], d_r[64:65, e, :],
                    start=True, stop=True,
                )
                nc.vector.reciprocal_approx_fast(out=rb[:, e, :], in_=db_ps)
            for e in range(2):
                if e == 0:
                    nc.vector.tensor_mul(
                        qT_t[0:64, hp, :], ps_o[e][0:64, :], rb[:, e, :])
                else:
                    o_n = opool.tile([64, QT], F32R, tag="onorm")
                    nc.vector.tensor_mul(o_n, ps_o[e][0:64, :], rb[:, e, :])
                    nc.sync.dma_start(out=qT_t[64:128, hp, :], in_=o_n)

        def out_proj(qt, qT_t):
            # y_part rows [512*qt, 512*qt+512) = out^T.T @ WpT  (partial sums)
            y_part = y_parts[qt]
            for i in range(QT // 128):
                for ct in range(C // QT):
                    ps = psP.tile([128, QT], F32, tag="ps", name="ps_y")
                    for cc in range(NDC):
                        nc.tensor.matmul(
                            ps,
                            qT_t[:, cc, i * 128:(i + 1) * 128],
                            wp_sb[:, cc, ct * QT:(ct + 1) * QT],
                            start=(cc == 0), stop=(cc == NDC - 1),
                        )
                    y_sb = ypool.tile([128, QT], F32, tag="ysb", name="y_sb")
                    nc.vector.tensor_copy(y_sb, ps)
                    nc.gpsimd.dma_start(
                        out=y_part[i * 128:(i + 1) * 128,
                                   ct * QT:(ct + 1) * QT],
                        in_=y_sb)

        def layer_norm(hc):
            # hc = half-chunk index (256 rows), reading y_reds[hc]
            y_red = y_reds[hc]
            ntn = QT // 256
            mv_all = lnpool.tile([128, ntn, 2], F32, tag="mv")
            # pass 1: stats (tiles are released after bn_stats)
            for i in range(ntn):
                stats = lnpool.tile([128, 2, 6], F32, tag="stats")
                for s in range(2):
                    y_h = ypool.tile([128, QT], F32, tag="ysb", name="y_h")
                    nc.scalar.dma_start(
                        out=y_h, in_=y_red[i * 128:(i + 1) * 128,
                                           s * QT:(s + 1) * QT])
                    nc.vector.bn_stats(out=stats[:, s, :], in_=y_h)
                nc.vector.bn_aggr(out=mv_all[:, i, :], in_=stats)
            # one Sqrt for all row-tiles of this chunk (fewer ACT table swaps)
            rstd = lnpool.tile([128, ntn], F32, tag="rstd")
            nc.scalar.activation(
                out=rstd, in_=mv_all[:, :, 1],
                func=mybir.ActivationFunctionType.Sqrt,
                bias=eps_sb, scale=1.0,
            )
            nc.vector.reciprocal_approx_fast(out=rstd, in_=rstd)
            # pass 2: reload, normalize, write out
            for i in range(ntn):
                tn = hc * 2 + i
                for s in range(2):
                    csl = slice(s * QT, (s + 1) * QT)
                    y_h = ypool.tile([128, QT], F32, tag="ysb", name="y_h2")
                    nc.scalar.dma_start(
                        out=y_h, in_=y_red[i * 128:(i + 1) * 128, csl])
                    nc.vector.tensor_scalar(
                        out=y_h, in0=y_h,
                        scalar1=mv_all[:, i, 0:1], scalar2=rstd[:, i:i + 1],
                        op0=mybir.AluOpType.subtract, op1=mybir.AluOpType.mult,
                    )
                    nc.vector.tensor_mul(y_h, y_h, gamma_sb[:, csl])
                    nc.vector.tensor_add(y_h, y_h, beta_sb[:, csl])
                    nc.gpsimd.dma_start(
                        out=y_out.ap()[tn * 128:(tn + 1) * 128, csl], in_=y_h)

        # ---- the fused pipeline over 512-row t-tiles ----
        for tt in range(NQT):
            if tt == 0:
                x_halves = x_first
            else:
                x_halves = []
                for xh in range(2):
                    x_h = xpool.tile([128, NIC, XH], F32R, name="x_h")
                    t0 = tt * QT + xh * XH
                    nc.scalar.dma_start(
                        out=x_h, in_=xT.ap()[:, t0:t0 + XH]
                        .rearrange("(a p) t -> p a t", p=128))
                    x_halves.append(x_h)
            qT_t = qpool.tile([128, NDC, QT], F32R, name="qT_t")
            project(tt, x_halves, qT_t)
            for hp in range(HPG // 2):
                attention(hp, tt, qT_t)
            out_proj(tt, qT_t)
            for half in range(2):
                hc = tt * 2 + half
                rs = slice(half * (QT // 2), (half + 1) * (QT // 2))
                nc.gpsimd.collective_compute(
                    "AllReduce",
                    mybir.AluOpType.add,
                    replica_groups=REPLICA_GROUPS,
                    ins=[y_parts[tt][rs, :]],
                    outs=[y_reds[hc]],
                )
                layer_norm(hc)

    persist.release()


_PROG = None


def _get_program():
    global _PROG
    if _PROG is None:
        _PROG = build_program()
    return _PROG


def _round_f32r(a):
    """Round fp32 to the f32r grid (11 explicit mantissa bits, RNE-ish)."""
    bits = np.ascontiguousarray(a, np.float32).view(np.uint32)
    r = ((bits.astype(np.uint64) + 0x800) & 0xFFFFF000).astype(np.uint32)
    return r.view(np.float32)


def make_in_maps(x, Wk, Wq, Wv, Wp, gamma, beta):
    import ml_dtypes
    x = np.asarray(x, dtype=np.float32)
    masks = np.zeros((4, KT, QT), dtype=np.float32)
    for j in range(4):
        k = np.arange(KT)[:, None]
        q = np.arange(QT)[None, :]
        masks[j] = (128 * j + k <= q).astype(np.float32)
    masks = masks.astype(ml_dtypes.bfloat16)
    in_maps = []
    for c in range(8):
        b, hg = c // HG, c % HG
        sl = slice(hg * CG, (hg + 1) * CG)
        in_maps.append({
            "xT": _round_f32r(x[b].T),
            "wqT": _round_f32r(np.asarray(Wq, np.float32)[sl, :].T),
            "wkT": _round_f32r(np.asarray(Wk, np.float32)[sl, :].T),
            "wvT": _round_f32r(np.asarray(Wv, np.float32)[sl, :].T),
            "wpT": _round_f32r(np.asarray(Wp, np.float32)[:, sl].T),
            "gamma": np.asarray(gamma, np.float32),
            "beta": np.asarray(beta, np.float32),
            "masks": masks,
        })
    return in_maps


def kernel(x, Wk, Wq, Wv, Wp, gamma, beta, _trace=False, _trace_kwargs=None):
    nc, io = _get_program()
    in_maps = make_in_maps(x, Wk, Wq, Wv, Wp, gamma, beta)
    res = run_bass_kernel_spmd(
        nc, in_maps, core_ids=list(range(8)),
        trace=_trace, **(_trace_kwargs or {}),
    )
    out = np.stack([res.results[HG * b]["y"] for b in range(B)])
    if _trace:
        kernel.last_results = res
    return out
